# revision 17
# baseline (speedup 1.0000x reference)
"""Trainium2 Bass kernel: fused multi-head attention block (projections +
softmax attention + output projection + residual + LayerNorm).

Sharding: 8 cores = 2 batches x 4 query-chunks of 512. Each core computes
K/V for its whole batch (replicated within the 4-core batch group), Q only
for its 512-query chunk, full attention for that chunk over all 16 heads,
the output projection, residual add and LayerNorm. No collectives.

All cores run the same program; per-core inputs are pre-sliced on the host
with the key/value token order ROTATED so the core's query chunk occupies
rows 0..511 (attention is permutation-invariant over keys, and the key
padding mask is rotated identically).

Device-side layouts (per core):
  xt   [1024, 2048] bf16  x[b] transposed (feature-major), rotated
  xq   [512, 1024]  f32   query-chunk rows of x[b] (residual input)
  wq/wk/wv [1024, 1024] bf16  [c, h*64] (head-minor)
  wo   [1024, 1024] bf16  [(h*64+d), m]
  bias [16, 128]    f32   additive key mask bias per key tile/partition
  gamma/beta [1024] bf16
Output: y [512, 1024] f32.
"""

import contextlib

import numpy as np
import ml_dtypes

import concourse.bass as bass
import concourse.tile as tile
from concourse import mybir
from concourse import bass_utils

BF16 = ml_dtypes.bfloat16
N_CORES = 8
B, L, D, H, DH = 2, 2048, 1024, 16, 64
Q = L // 4          # queries per core
CT = D // 128       # contraction tiles over features
JT = L // 128       # key tiles
IT = Q // 128       # query tiles
LN_EPS = 1e-5

F32 = mybir.dt.float32
BF = mybir.dt.bfloat16


def _split_waits(nc, maxw=1):
    """This walrus build rejects instructions with more than one sync wait;
    split excess waits into preceding NOPs on the same engine."""
    ctr = 0
    for fn in nc.m.functions:
        for bb in fn.blocks:
            new_insts = []
            for inst in bb.instructions:
                si = inst.sync_info
                if si is not None and len(si.on_wait) > maxw:
                    waits = list(si.on_wait)
                    excess, keep = waits[:-maxw], waits[-maxw:]
                    for i in range(0, len(excess), maxw):
                        ctr += 1
                        new_insts.append(mybir.InstNoOp(
                            name=f"waitsplit_nop_{ctr}",
                            engine=inst.engine,
                            sync_info=mybir.SyncInfo(
                                on_wait=excess[i:i + maxw], on_update=[]),
                            text_hint="waitsplit",
                        ))
                    si.on_wait = keep
                new_insts.append(inst)
            bb.instructions = new_insts
    return ctr


def _bcast_parts(ap, parts):
    """Broadcast-read view of a [1, N] AP as [1, parts, N] via a stride-0
    free dim (SBUF APs may not have a stride-0 partition dim)."""
    return bass.AP(tensor=ap.tensor, offset=ap.offset,
                   ap=[list(ap.ap)[0], [0, parts]] + list(ap.ap)[1:])


def _emit(nc, tc, hh, masked):
    Exp = mybir.ActivationFunctionType.Exp
    Sqrt = mybir.ActivationFunctionType.Sqrt

    xt_ap = hh["xt"].ap().rearrange("(t p) l -> p t l", p=128)      # [128,8,2048]
    wq_ap = hh["wq"].ap().rearrange("(t p) d -> p t d", p=128)
    wk_ap = hh["wk"].ap().rearrange("(t p) d -> p t d", p=128)
    wv_ap = hh["wv"].ap().rearrange("(t p) d -> p t d", p=128)
    wo_ap = hh["wo"].ap().rearrange("(t p) d -> p t d", p=128)
    bias_ap = hh["bias"].ap().rearrange("a b -> b a")               # [128,16]
    xq_ap = hh["xq"].ap()
    y_ap = hh["y"].ap()

    def bcast_dram(h1d, parts=128):
        a = h1d.ap()
        return bass.AP(tensor=a.tensor, offset=a.offset,
                       ap=[[0, parts]] + list(a.ap))

    with contextlib.ExitStack() as ctx:
        const = ctx.enter_context(tc.tile_pool(name="const", bufs=1))
        wpool = ctx.enter_context(tc.tile_pool(name="wpool", bufs=2))
        xtp = ctx.enter_context(tc.tile_pool(name="xtp", bufs=2))
        expp = ctx.enter_context(tc.tile_pool(name="expp", bufs=2))
        ktp = ctx.enter_context(tc.tile_pool(name="ktp", bufs=3))
        vp = ctx.enter_context(tc.tile_pool(name="vp", bufs=1))
        qtp = ctx.enter_context(tc.tile_pool(name="qtp", bufs=1))
        ptp = ctx.enter_context(tc.tile_pool(name="ptp", bufs=1))
        npool = ctx.enter_context(tc.tile_pool(name="npool", bufs=3))
        xqp = ctx.enter_context(tc.tile_pool(name="xqp", bufs=1))
        lnp = ctx.enter_context(tc.tile_pool(name="lnp", bufs=3))
        statp = ctx.enter_context(tc.tile_pool(name="statp", bufs=4))
        psS = ctx.enter_context(tc.tile_pool(name="psS", bufs=2, space="PSUM"))
        psP = ctx.enter_context(tc.tile_pool(name="psP", bufs=2, space="PSUM"))
        psD = ctx.enter_context(tc.tile_pool(name="psD", bufs=2, space="PSUM"))

        # ---- constants / small loads ----
        eps_sb = const.tile([128, 1], F32)
        nc.vector.memset(eps_sb[:], LN_EPS)
        ones64 = const.tile([1, 64], F32)
        nc.vector.memset(ones64[:], 1.0)
        bias_sb = const.tile([128, 16], F32)
        nc.gpsimd.dma_start(out=bias_sb[:], in_=bias_ap)
        gamma_sb = const.tile([128, 1024], BF)
        nc.gpsimd.dma_start(out=gamma_sb[:], in_=bcast_dram(hh["gamma"]))
        beta_sb = const.tile([128, 1024], BF)
        nc.gpsimd.dma_start(out=beta_sb[:], in_=bcast_dram(hh["beta"]))

        # ---- big SBUF tensors ----
        v_all = vp.tile([128, JT, H, DH + 1], BF)  # V by key tile, +ones col
        qt_all = qtp.tile([128, 8, Q], BF)         # Q^T for the chunk
        probt = ptp.tile([128, 8, Q], BF)          # normalized P^T stacked

        nc.vector.memset(v_all[:, :, :, DH:DH + 1], 1.0)

        # weights streamed through wpool: wv, wq first; wk, wo reuse slots
        # (weights go on the ACT HWDGE queue, xt on the SP queue, so the
        # two initial load streams run in parallel)
        xt0 = xtp.tile([128, 4, 2048], BF, tag="xt")
        nc.sync.dma_start(out=xt0[:], in_=xt_ap[:, 0:4, :])
        wv_sb = wpool.tile([128, 8, 1024], BF, tag="w")
        nc.scalar.dma_start(out=wv_sb[:], in_=wv_ap)
        xt1 = xtp.tile([128, 4, 2048], BF, tag="xt")
        nc.scalar.dma_start(out=xt1[:], in_=xt_ap[:, 4:8, :])
        wq_sb = wpool.tile([128, 8, 1024], BF, tag="w")
        nc.sync.dma_start(out=wq_sb[:], in_=wq_ap)

        def xt_sl(ct, lo, size):
            t = xt0 if ct < 4 else xt1
            return t[:, ct % 4, lo:lo + size]

        # ---- V projection: [token 128][h*64] ----
        for lt in range(JT):
            ps = psS.tile([128, 2, 512], F32, tag="ss")
            for nt in range(2):
                for ct in range(CT):
                    nc.tensor.matmul(
                        ps[:, nt, :], xt_sl(ct, lt * 128, 128),
                        wv_sb[:, ct, nt * 512:(nt + 1) * 512],
                        start=(ct == 0), stop=(ct == CT - 1))
            nc.vector.tensor_copy(
                v_all[:, lt, :, 0:DH],
                ps.rearrange("p n (h d) -> p (n h) d", h=8))

        wk_sb = wpool.tile([128, 8, 1024], BF, tag="w")
        nc.scalar.dma_start(out=wk_sb[:], in_=wk_ap)

        # ---- Q^T projection: [d' 128][i 512] ----
        for dtp in range(4):
            ps = psS.tile([128, 2, 512], F32, tag="ss")
            for half in range(2):
                dt = 2 * dtp + half
                for ct in range(CT):
                    nc.tensor.matmul(
                        ps[:, half, :], wq_sb[:, ct, dt * 128:(dt + 1) * 128],
                        xt_sl(ct, 0, Q),
                        start=(ct == 0), stop=(ct == CT - 1))
            nc.vector.tensor_copy(qt_all[:, 2 * dtp:2 * dtp + 2, :], ps[:])

        wo_sb = wpool.tile([128, 8, 1024], BF, tag="w")
        nc.scalar.dma_start(out=wo_sb[:], in_=wo_ap)

        # ---- K^T projection (per d'-tile) interleaved with attention ----
        for dt in range(8):
            kt_t = ktp.tile([128, 2048], BF, tag="kt")
            for jp in range(2):
                ps = psS.tile([128, 2, 512], F32, tag="ss")
                for j4 in range(2):
                    for ct in range(CT):
                        nc.tensor.matmul(
                            ps[:, j4, :],
                            wk_sb[:, ct, dt * 128:(dt + 1) * 128],
                            xt_sl(ct, jp * 1024 + j4 * 512, 512),
                            start=(ct == 0), stop=(ct == CT - 1))
                nc.vector.tensor_copy(
                    kt_t[:, jp * 1024:(jp + 1) * 1024], ps[:])

            # Both heads of this d'-tile run as concurrent row-group
            # matmuls (K=64 in rows 0-63 and 64-127 of the PE array),
            # sharing one batched exp per key tile.
            expt_halves = []
            for half in range(2):
                expt = expp.tile([128, JT // 2, 2, 512], BF, tag="e")
                expt_halves.append(expt)
                for jj in range(JT // 2):
                    jt = half * (JT // 2) + jj
                    ps = psS.tile([128, 2, 512], F32, tag="ss")
                    for hb in range(2):
                        nc.tensor.matmul(
                            ps[:, hb, :],
                            kt_t[hb * 64:hb * 64 + 64,
                                 jt * 128:(jt + 1) * 128],
                            qt_all[hb * 64:hb * 64 + 64, dt, :],
                            start=True, stop=True)
                    if masked:
                        for hb in range(2):
                            nc.scalar.activation(
                                expt[:, jj, hb, :], ps[:, hb, :], Exp,
                                bias=bias_sb[:, jt:jt + 1], scale=1.0 / 8.0)
                    else:
                        nc.scalar.activation(
                            expt[:, jj, :, :], ps[:], Exp,
                            bias=0.0, scale=1.0 / 8.0)
            for hb in range(2):
                h, poff = 2 * dt + hb, hb * 64
                ps_p = psP.tile([DH + 1, 512], F32, tag="pp")
                for jt in range(JT):
                    nc.tensor.matmul(
                        ps_p[:], v_all[:, jt, h, 0:DH + 1],
                        expt_halves[jt // (JT // 2)][:, jt % (JT // 2), hb, :],
                        start=(jt == 0), stop=(jt == JT - 1))
                den = npool.tile([1, 512], F32, tag="n")
                nc.vector.tensor_copy(den[:], ps_p[DH:DH + 1, :])
                ps_d = psD.tile([64, 512], F32, tag="dd")
                nc.tensor.matmul(ps_d[:], ones64[:], den[:],
                                 start=True, stop=True)
                rdiv = npool.tile([64, 512], F32, tag="n")
                nc.vector.reciprocal(rdiv[:], ps_d[:])
                nc.vector.tensor_mul(
                    probt[poff:poff + 64, dt, :], ps_p[0:DH, :], rdiv[:])

        # ---- output projection + residual + LayerNorm ----
        for it in range(IT):
            xq_t = xqp.tile([128, 1024], F32, tag="xq")
            nc.sync.dma_start(out=xq_t[:],
                              in_=xq_ap[it * 128:(it + 1) * 128, :])
            ps_r = psS.tile([128, 2, 512], F32, tag="ss")
            for mh in range(2):
                for kt in range(8):
                    nc.tensor.matmul(
                        ps_r[:, mh, :],
                        probt[:, kt, it * 128:(it + 1) * 128],
                        wo_sb[:, kt, mh * 512:(mh + 1) * 512],
                        start=(kt == 0), stop=(kt == 7))
            h_sb = lnp.tile([128, 1024], F32, tag="ln")
            nc.vector.tensor_add(h_sb[:], ps_r.rearrange("p a b -> p (a b)"),
                                 xq_t[:])
            stats = statp.tile([128, 2, 6], F32)
            nc.vector.bn_stats(stats[:, 0, :], h_sb[:, 0:512])
            nc.vector.bn_stats(stats[:, 1, :], h_sb[:, 512:1024])
            mv = statp.tile([128, 2], F32)
            nc.vector.bn_aggr(mv[:], stats[:])
            std = statp.tile([128, 1], F32)
            nc.scalar.activation(std[:], mv[:, 1:2], Sqrt,
                                 bias=eps_sb[:], scale=1.0)
            rstd = statp.tile([128, 1], F32)
            nc.vector.reciprocal(rstd[:], std[:])
            t1 = lnp.tile([128, 1024], F32, tag="ln")
            nc.vector.tensor_scalar(
                t1[:], h_sb[:], mv[:, 0:1], rstd[:],
                op0=mybir.AluOpType.subtract, op1=mybir.AluOpType.mult)
            t2 = lnp.tile([128, 1024], F32, tag="ln")
            nc.vector.tensor_mul(t2[:], t1[:], gamma_sb[:])
            out_t = lnp.tile([128, 1024], F32, tag="ln")
            nc.vector.tensor_add(out_t[:], t2[:], beta_sb[:])
            nc.sync.dma_start(y_ap[it * 128:(it + 1) * 128, :], out_t[:])


def build_module(split=True, masked=False):
    nc = bass.Bass("TRN2", target_bir_lowering=False, debug=False,
                   num_devices=N_CORES)
    hh = {
        "xt": nc.dram_tensor("xt", [D, L], BF, kind="ExternalInput"),
        "xq": nc.dram_tensor("xq", [Q, D], F32, kind="ExternalInput"),
        "wq": nc.dram_tensor("wq", [D, D], BF, kind="ExternalInput"),
        "wk": nc.dram_tensor("wk", [D, D], BF, kind="ExternalInput"),
        "wv": nc.dram_tensor("wv", [D, D], BF, kind="ExternalInput"),
        "wo": nc.dram_tensor("wo", [D, D], BF, kind="ExternalInput"),
        "bias": nc.dram_tensor("bias", [16, 128], F32, kind="ExternalInput"),
        "gamma": nc.dram_tensor("gamma", [D], BF, kind="ExternalInput"),
        "beta": nc.dram_tensor("beta", [D], BF, kind="ExternalInput"),
        "y": nc.dram_tensor("y", [Q, D], F32, kind="ExternalOutput"),
    }
    with tile.TileContext(nc) as tc:
        _emit(nc, tc, hh, masked)
    if split:
        _split_waits(nc, 1)
    return nc


_CACHE = {}


def get_module(masked=False):
    key = ("nc", masked)
    if key not in _CACHE:
        _CACHE[key] = build_module(masked=masked)
    return _CACHE[key]


def prep_inputs(x, mask, w_q, w_k, w_v, w_o, ln_gamma, ln_beta):
    x = np.asarray(x, dtype=np.float32)
    mask = np.asarray(mask)
    shared = {
        "wq": np.ascontiguousarray(
            np.asarray(w_q, np.float32).transpose(1, 0, 2).reshape(D, D)
        ).astype(BF16),
        "wk": np.ascontiguousarray(
            np.asarray(w_k, np.float32).transpose(1, 0, 2).reshape(D, D)
        ).astype(BF16),
        "wv": np.ascontiguousarray(
            np.asarray(w_v, np.float32).transpose(1, 0, 2).reshape(D, D)
        ).astype(BF16),
        "wo": np.asarray(w_o, np.float32).reshape(D, D).astype(BF16),
        "gamma": np.asarray(ln_gamma, np.float32).astype(BF16),
        "beta": np.asarray(ln_beta, np.float32).astype(BF16),
    }
    in_maps = []
    for c in range(N_CORES):
        b, q0 = c // 4, (c % 4) * Q
        perm = np.r_[q0:L, 0:q0]
        xb = x[b][perm]                       # rotated: q-chunk first
        m = {
            "xt": np.ascontiguousarray(xb.T).astype(BF16),
            "xq": np.ascontiguousarray(x[b, q0:q0 + Q, :]),
            "bias": np.where(mask[b][perm], 0.0, -1e9).astype(
                np.float32).reshape(16, 128),
        }
        m.update(shared)
        in_maps.append(m)
    masked = not bool(mask.all())
    return in_maps, masked


def assemble(results):
    out = np.empty((B, L, D), dtype=np.float32)
    for c in range(N_CORES):
        b, q0 = c // 4, (c % 4) * Q
        out[b, q0:q0 + Q, :] = results[c]["y"]
    return out


def run(in_maps, masked=False, **kwargs):
    nc = get_module(masked)
    return bass_utils.run_bass_kernel_spmd(
        nc, in_maps, core_ids=list(range(N_CORES)), **kwargs)


def kernel(x, mask, w_q, w_k, w_v, w_o, ln_gamma, ln_beta):
    in_maps, masked = prep_inputs(x, mask, w_q, w_k, w_v, w_o,
                                  ln_gamma, ln_beta)
    res = run(in_maps, masked)
    return assemble(res.results)


# revision 24
# speedup vs baseline: 1.0253x; 1.0253x over previous
"""Trainium2 Bass kernel: fused multi-head attention block (projections +
softmax attention + output projection + residual + LayerNorm).

Sharding: 8 cores = 2 batches x 4 query-chunks of 512. Each core computes
K/V for its whole batch (replicated within the 4-core batch group), Q only
for its 512-query chunk, full attention for that chunk over all 16 heads,
the output projection, residual add and LayerNorm. No collectives.

All cores run the same program; per-core inputs are pre-sliced on the host
with the key/value token order ROTATED so the core's query chunk occupies
rows 0..511 (attention is permutation-invariant over keys, and the key
padding mask is rotated identically).

Device-side layouts (per core):
  xt   [1024, 2048] bf16  x[b] transposed (feature-major), rotated
  xq   [512, 1024]  f32   query-chunk rows of x[b] (residual input)
  wq/wk/wv [1024, 1024] bf16  [c, h*64] (head-minor)
  wo   [1024, 1024] bf16  [(h*64+d), m]
  bias [16, 128]    f32   additive key mask bias per key tile/partition
  gamma/beta [1024] bf16
Output: y [512, 1024] f32.
"""

import contextlib

import numpy as np
import ml_dtypes

import concourse.bass as bass
import concourse.tile as tile
from concourse import mybir
from concourse import bass_utils

BF16 = ml_dtypes.bfloat16
N_CORES = 8
B, L, D, H, DH = 2, 2048, 1024, 16, 64
Q = L // 4          # queries per core
CT = D // 128       # contraction tiles over features
JT = L // 128       # key tiles
IT = Q // 128       # query tiles
LN_EPS = 1e-5

F32 = mybir.dt.float32
BF = mybir.dt.bfloat16


def _split_waits(nc, maxw=1):
    """This walrus build rejects instructions with more than one sync wait;
    split excess waits into preceding NOPs on the same engine."""
    ctr = 0
    for fn in nc.m.functions:
        for bb in fn.blocks:
            new_insts = []
            for inst in bb.instructions:
                si = inst.sync_info
                if si is not None and len(si.on_wait) > maxw:
                    waits = list(si.on_wait)
                    excess, keep = waits[:-maxw], waits[-maxw:]
                    for i in range(0, len(excess), maxw):
                        ctr += 1
                        new_insts.append(mybir.InstNoOp(
                            name=f"waitsplit_nop_{ctr}",
                            engine=inst.engine,
                            sync_info=mybir.SyncInfo(
                                on_wait=excess[i:i + maxw], on_update=[]),
                            text_hint="waitsplit",
                        ))
                    si.on_wait = keep
                new_insts.append(inst)
            bb.instructions = new_insts
    return ctr


def _bcast_parts(ap, parts):
    """Broadcast-read view of a [1, N] AP as [1, parts, N] via a stride-0
    free dim (SBUF APs may not have a stride-0 partition dim)."""
    return bass.AP(tensor=ap.tensor, offset=ap.offset,
                   ap=[list(ap.ap)[0], [0, parts]] + list(ap.ap)[1:])


def _emit(nc, tc, hh, masked):
    Exp = mybir.ActivationFunctionType.Exp
    Sqrt = mybir.ActivationFunctionType.Sqrt

    xt_ap = hh["xt"].ap().rearrange("(t p) l -> p t l", p=128)      # [128,8,2048]
    wq_ap = hh["wq"].ap().rearrange("(t p) d -> p t d", p=128)
    wk_ap = hh["wk"].ap().rearrange("(t p) d -> p t d", p=128)
    wv_ap = hh["wv"].ap().rearrange("(t p) d -> p t d", p=128)
    wo_ap = hh["wo"].ap().rearrange("(t p) d -> p t d", p=128)
    bias_ap = hh["bias"].ap().rearrange("a b -> b a")               # [128,16]
    xq_ap = hh["xq"].ap()
    y_ap = hh["y"].ap()

    def bcast_dram(h1d, parts=128):
        a = h1d.ap()
        return bass.AP(tensor=a.tensor, offset=a.offset,
                       ap=[[0, parts]] + list(a.ap))

    with contextlib.ExitStack() as ctx:
        const = ctx.enter_context(tc.tile_pool(name="const", bufs=1))
        wpool = ctx.enter_context(tc.tile_pool(name="wpool", bufs=2))
        xtp = ctx.enter_context(tc.tile_pool(name="xtp", bufs=2))
        expp = ctx.enter_context(tc.tile_pool(name="expp", bufs=2))
        ktp = ctx.enter_context(tc.tile_pool(name="ktp", bufs=3))
        vp = ctx.enter_context(tc.tile_pool(name="vp", bufs=1))
        qtp = ctx.enter_context(tc.tile_pool(name="qtp", bufs=1))
        ptp = ctx.enter_context(tc.tile_pool(name="ptp", bufs=1))
        npool = ctx.enter_context(tc.tile_pool(name="npool", bufs=3))
        xqp = ctx.enter_context(tc.tile_pool(name="xqp", bufs=1))
        lnp = ctx.enter_context(tc.tile_pool(name="lnp", bufs=3))
        statp = ctx.enter_context(tc.tile_pool(name="statp", bufs=4))
        psS = ctx.enter_context(tc.tile_pool(name="psS", bufs=2, space="PSUM"))
        psP = ctx.enter_context(tc.tile_pool(name="psP", bufs=2, space="PSUM"))
        psD = ctx.enter_context(tc.tile_pool(name="psD", bufs=2, space="PSUM"))

        # ---- constants / small loads ----
        eps_sb = const.tile([128, 1], F32)
        nc.vector.memset(eps_sb[:], LN_EPS)
        ones64 = const.tile([1, 64], F32)
        nc.vector.memset(ones64[:], 1.0)
        bias_sb = const.tile([128, 16], F32)
        nc.gpsimd.dma_start(out=bias_sb[:], in_=bias_ap)
        gamma_sb = const.tile([128, 1024], BF)
        beta_sb = const.tile([128, 1024], BF)

        # ---- big SBUF tensors ----
        v_all = vp.tile([128, JT, H, DH + 1], BF)  # V by key tile, +ones col
        qt_all = qtp.tile([128, 8, Q], BF)         # Q^T for the chunk
        probt = ptp.tile([128, 8, Q], BF)          # normalized P^T stacked

        nc.vector.memset(v_all[:, :, :, DH:DH + 1], 1.0)

        # weights streamed through wpool: wv, wq first; wk, wo reuse slots
        # (weights go on the ACT HWDGE queue, xt on the SP queue, so the
        # two initial load streams run in parallel)
        xt0 = xtp.tile([128, 4, 2048], BF, tag="xt")
        nc.sync.dma_start(out=xt0[:], in_=xt_ap[:, 0:4, :])
        wv_sb = wpool.tile([128, 8, 1024], BF, tag="w")
        nc.scalar.dma_start(out=wv_sb[:], in_=wv_ap)
        xt1 = xtp.tile([128, 4, 2048], BF, tag="xt")
        nc.scalar.dma_start(out=xt1[:], in_=xt_ap[:, 4:8, :])
        wq_sb = wpool.tile([128, 8, 1024], BF, tag="w")
        nc.sync.dma_start(out=wq_sb[:], in_=wq_ap)

        def xt_sl(ct, lo, size):
            t = xt0 if ct < 4 else xt1
            return t[:, ct % 4, lo:lo + size]

        # ---- V projection: [token 128][h*64] ----
        for lt in range(JT):
            ps = psS.tile([128, 2, 512], F32, tag="ss")
            for nt in range(2):
                for ct in range(CT):
                    nc.tensor.matmul(
                        ps[:, nt, :], xt_sl(ct, lt * 128, 128),
                        wv_sb[:, ct, nt * 512:(nt + 1) * 512],
                        start=(ct == 0), stop=(ct == CT - 1))
            nc.vector.tensor_copy(
                v_all[:, lt, :, 0:DH],
                ps.rearrange("p n (h d) -> p (n h) d", h=8))

        wk_sb = wpool.tile([128, 8, 1024], BF, tag="w")
        nc.scalar.dma_start(out=wk_sb[:], in_=wk_ap)

        # ---- Q^T projection: [d' 128][i 512] ----
        for dtp in range(4):
            ps = psS.tile([128, 2, 512], F32, tag="ss")
            for half in range(2):
                dt = 2 * dtp + half
                for ct in range(CT):
                    nc.tensor.matmul(
                        ps[:, half, :], wq_sb[:, ct, dt * 128:(dt + 1) * 128],
                        xt_sl(ct, 0, Q),
                        start=(ct == 0), stop=(ct == CT - 1))
            nc.vector.tensor_copy(qt_all[:, 2 * dtp:2 * dtp + 2, :], ps[:])

        wo_sb = wpool.tile([128, 8, 1024], BF, tag="w")
        nc.scalar.dma_start(out=wo_sb[:], in_=wo_ap)

        # ---- K^T projection (per d'-tile) interleaved with attention ----
        for dt in range(8):
            kt_t = ktp.tile([128, 2048], BF, tag="kt")
            for jp in range(2):
                ps = psS.tile([128, 2, 512], F32, tag="ss")
                for j4 in range(2):
                    for ct in range(CT):
                        nc.tensor.matmul(
                            ps[:, j4, :],
                            wk_sb[:, ct, dt * 128:(dt + 1) * 128],
                            xt_sl(ct, jp * 1024 + j4 * 512, 512),
                            start=(ct == 0), stop=(ct == CT - 1))
                if jp == 0:
                    nc.vector.tensor_copy(
                        kt_t[:, jp * 1024:(jp + 1) * 1024], ps[:])
                else:
                    nc.scalar.copy(
                        kt_t[:, jp * 1024:(jp + 1) * 1024], ps[:])

            # Both heads of this d'-tile run as concurrent row-group
            # matmuls (K=64 in rows 0-63 and 64-127 of the PE array),
            # sharing one batched exp per key tile.
            expt_halves = []
            for half in range(2):
                expt = expp.tile([128, JT // 2, 2, 512], BF, tag="e")
                expt_halves.append(expt)
                for jj in range(JT // 2):
                    jt = half * (JT // 2) + jj
                    ps = psS.tile([128, 2, 512], F32, tag="ss")
                    for hb in range(2):
                        nc.tensor.matmul(
                            ps[:, hb, :],
                            kt_t[hb * 64:hb * 64 + 64,
                                 jt * 128:(jt + 1) * 128],
                            qt_all[hb * 64:hb * 64 + 64, dt, :],
                            start=True, stop=True)
                    if masked:
                        for hb in range(2):
                            nc.scalar.activation(
                                expt[:, jj, hb, :], ps[:, hb, :], Exp,
                                bias=bias_sb[:, jt:jt + 1], scale=1.0 / 8.0)
                    else:
                        nc.scalar.activation(
                            expt[:, jj, :, :], ps[:], Exp,
                            bias=0.0, scale=1.0 / 8.0)
            # PV: interleave the two heads' accumulation chains so the PE
            # alternates PSUM banks (hides the same-bank drain latency).
            pv_ps = [psP.tile([DH + 1, 512], F32, tag="pp", name=f"pv{hb}")
                     for hb in range(2)]
            for jt in range(JT):
                for hb in range(2):
                    nc.tensor.matmul(
                        pv_ps[hb][:], v_all[:, jt, 2 * dt + hb, 0:DH + 1],
                        expt_halves[jt // (JT // 2)][:, jt % (JT // 2), hb, :],
                        start=(jt == 0), stop=(jt == JT - 1))
            for hb in range(2):
                poff = hb * 64
                ps_p = pv_ps[hb]
                den = npool.tile([1, 512], F32, tag="n")
                nc.scalar.copy(den[:], ps_p[DH:DH + 1, :])
                ps_d = psD.tile([64, 512], F32, tag="dd")
                nc.tensor.matmul(ps_d[:], ones64[:], den[:],
                                 start=True, stop=True)
                rdiv = npool.tile([64, 512], F32, tag="n")
                nc.vector.reciprocal(rdiv[:], ps_d[:])
                nc.vector.tensor_mul(
                    probt[poff:poff + 64, dt, :], ps_p[0:DH, :], rdiv[:])

        # ---- output projection + residual + LayerNorm ----
        nc.gpsimd.dma_start(out=gamma_sb[:], in_=bcast_dram(hh["gamma"]))
        nc.gpsimd.dma_start(out=beta_sb[:], in_=bcast_dram(hh["beta"]))
        for it in range(IT):
            xq_t = xqp.tile([128, 1024], F32, tag="xq")
            nc.sync.dma_start(out=xq_t[:],
                              in_=xq_ap[it * 128:(it + 1) * 128, :])
            ps_r = psS.tile([128, 2, 512], F32, tag="ss")
            for mh in range(2):
                for kt in range(8):
                    nc.tensor.matmul(
                        ps_r[:, mh, :],
                        probt[:, kt, it * 128:(it + 1) * 128],
                        wo_sb[:, kt, mh * 512:(mh + 1) * 512],
                        start=(kt == 0), stop=(kt == 7))
            h_sb = lnp.tile([128, 1024], F32, tag="ln")
            nc.vector.tensor_add(h_sb[:], ps_r.rearrange("p a b -> p (a b)"),
                                 xq_t[:])
            stats = statp.tile([128, 2, 6], F32)
            nc.vector.bn_stats(stats[:, 0, :], h_sb[:, 0:512])
            nc.vector.bn_stats(stats[:, 1, :], h_sb[:, 512:1024])
            mv = statp.tile([128, 2], F32)
            nc.vector.bn_aggr(mv[:], stats[:])
            std = statp.tile([128, 1], F32)
            nc.scalar.activation(std[:], mv[:, 1:2], Sqrt,
                                 bias=eps_sb[:], scale=1.0)
            rstd = statp.tile([128, 1], F32)
            nc.vector.reciprocal(rstd[:], std[:])
            t1 = lnp.tile([128, 1024], F32, tag="ln")
            nc.vector.tensor_scalar(
                t1[:], h_sb[:], mv[:, 0:1], rstd[:],
                op0=mybir.AluOpType.subtract, op1=mybir.AluOpType.mult)
            t2 = lnp.tile([128, 1024], F32, tag="ln")
            nc.vector.tensor_mul(t2[:], t1[:], gamma_sb[:])
            out_t = lnp.tile([128, 1024], F32, tag="ln")
            nc.vector.tensor_add(out_t[:], t2[:], beta_sb[:])
            nc.sync.dma_start(y_ap[it * 128:(it + 1) * 128, :], out_t[:])


def build_module(split=True, masked=False):
    nc = bass.Bass("TRN2", target_bir_lowering=False, debug=False,
                   num_devices=N_CORES)
    hh = {
        "xt": nc.dram_tensor("xt", [D, L], BF, kind="ExternalInput"),
        "xq": nc.dram_tensor("xq", [Q, D], F32, kind="ExternalInput"),
        "wq": nc.dram_tensor("wq", [D, D], BF, kind="ExternalInput"),
        "wk": nc.dram_tensor("wk", [D, D], BF, kind="ExternalInput"),
        "wv": nc.dram_tensor("wv", [D, D], BF, kind="ExternalInput"),
        "wo": nc.dram_tensor("wo", [D, D], BF, kind="ExternalInput"),
        "bias": nc.dram_tensor("bias", [16, 128], F32, kind="ExternalInput"),
        "gamma": nc.dram_tensor("gamma", [D], BF, kind="ExternalInput"),
        "beta": nc.dram_tensor("beta", [D], BF, kind="ExternalInput"),
        "y": nc.dram_tensor("y", [Q, D], F32, kind="ExternalOutput"),
    }
    with tile.TileContext(nc) as tc:
        _emit(nc, tc, hh, masked)
    if split:
        _split_waits(nc, 1)
    return nc


_CACHE = {}


def get_module(masked=False):
    key = ("nc", masked)
    if key not in _CACHE:
        _CACHE[key] = build_module(masked=masked)
    return _CACHE[key]


def prep_inputs(x, mask, w_q, w_k, w_v, w_o, ln_gamma, ln_beta):
    x = np.asarray(x, dtype=np.float32)
    mask = np.asarray(mask)
    shared = {
        "wq": np.ascontiguousarray(
            np.asarray(w_q, np.float32).transpose(1, 0, 2).reshape(D, D)
        ).astype(BF16),
        "wk": np.ascontiguousarray(
            np.asarray(w_k, np.float32).transpose(1, 0, 2).reshape(D, D)
        ).astype(BF16),
        "wv": np.ascontiguousarray(
            np.asarray(w_v, np.float32).transpose(1, 0, 2).reshape(D, D)
        ).astype(BF16),
        "wo": np.asarray(w_o, np.float32).reshape(D, D).astype(BF16),
        "gamma": np.asarray(ln_gamma, np.float32).astype(BF16),
        "beta": np.asarray(ln_beta, np.float32).astype(BF16),
    }
    in_maps = []
    for c in range(N_CORES):
        b, q0 = c // 4, (c % 4) * Q
        perm = np.r_[q0:L, 0:q0]
        xb = x[b][perm]                       # rotated: q-chunk first
        m = {
            "xt": np.ascontiguousarray(xb.T).astype(BF16),
            "xq": np.ascontiguousarray(x[b, q0:q0 + Q, :]),
            "bias": np.where(mask[b][perm], 0.0, -1e9).astype(
                np.float32).reshape(16, 128),
        }
        m.update(shared)
        in_maps.append(m)
    masked = not bool(mask.all())
    return in_maps, masked


def assemble(results):
    out = np.empty((B, L, D), dtype=np.float32)
    for c in range(N_CORES):
        b, q0 = c // 4, (c % 4) * Q
        out[b, q0:q0 + Q, :] = results[c]["y"]
    return out


def run(in_maps, masked=False, **kwargs):
    nc = get_module(masked)
    return bass_utils.run_bass_kernel_spmd(
        nc, in_maps, core_ids=list(range(N_CORES)), **kwargs)


def kernel(x, mask, w_q, w_k, w_v, w_o, ln_gamma, ln_beta):
    in_maps, masked = prep_inputs(x, mask, w_q, w_k, w_v, w_o,
                                  ln_gamma, ln_beta)
    res = run(in_maps, masked)
    return assemble(res.results)
